# revision 1
# baseline (speedup 1.0000x reference)
"""Trainium2 Bass kernel for nn_LocalConnectivity (diamond-ring circular stencil).

out[i,j] = sum_{d=1..5} w_d * sum_{|di|+|dj|=d} x[(i+di)%H, (j+dj)%W]

Strategy: row-shard across 8 NeuronCores (512 rows each + 5-row circular
halo, columns pre-padded with 5-col circular halo on host). Per core the
61-tap stencil is computed on the TensorEngine as 11 banded matmuls (one
per column shift dj in [-5,5]): PSUM[m, c] += W_dj[k, m] * strip[k, c+5+dj]
where W_dj is a [128, 118] constant band matrix holding the vertical taps
for that dj and the column shift rides the rhs access pattern for free.
float32r matmuls stream at 1 cycle/row (vs 4 for float32) at ~2e-4 rel err.
"""
import numpy as np
from contextlib import ExitStack

import concourse.bass as bass
import concourse.tile as tile
from concourse import bacc, mybir
from concourse.bass_utils import run_bass_kernel_spmd

N_CORES = 8
H = W = 4096
MAXD = 5
ROWS_PER_CORE = H // N_CORES          # 512
IN_ROWS = ROWS_PER_CORE + 2 * MAXD    # 522
IN_COLS = W + 2 * MAXD                # 4106
NCOL = 512                            # matmul free dim (one PSUM bank, fp32 max)
NCHUNK = W // NCOL                    # 8
M_OUT = 118                           # output rows per row-window (K=128 - 2*MAXD)
# row windows: (input_row_start, out_row_start, K, M)
WINDOWS = []
_o = 0
while _o < ROWS_PER_CORE:
    m = min(M_OUT, ROWS_PER_CORE - _o)
    WINDOWS.append((_o, _o, m + 2 * MAXD, m))
    _o += m

_CACHE = {}


def _band_weights(distance_weights: np.ndarray) -> np.ndarray:
    """w_flat [128, 11*118]: w_flat[k, (dj+5)*118 + m] = K2d[k-m-5, dj]."""
    wd = np.asarray(distance_weights, dtype=np.float32)
    w = np.zeros((11, 128, M_OUT), dtype=np.float32)
    for dj in range(-MAXD, MAXD + 1):
        for di in range(-MAXD, MAXD + 1):
            d = abs(di) + abs(dj)
            if not (1 <= d <= MAXD):
                continue
            m = np.arange(M_OUT)
            k = m + MAXD + di
            ok = (k >= 0) & (k < 128)
            w[dj + MAXD, k[ok], m[ok]] = wd[d - 1]
    return np.ascontiguousarray(w.transpose(1, 0, 2).reshape(128, 11 * M_OUT))


def _build():
    dtr = mybir.dt.float32r
    dtf = mybir.dt.float32
    nc = bacc.Bacc("TRN2", target_bir_lowering=False, debug=False,
                   num_devices=N_CORES)
    x = nc.dram_tensor("x", [IN_ROWS, IN_COLS], dtr, kind="ExternalInput").ap()
    wts = nc.dram_tensor("w", [128, 11 * M_OUT], dtr, kind="ExternalInput").ap()
    y = nc.dram_tensor("y", [ROWS_PER_CORE, W], dtf, kind="ExternalOutput").ap()

    with tile.TileContext(nc) as tc, ExitStack() as ctx:
        spool = ctx.enter_context(tc.tile_pool(name="strip", bufs=3))
        wpool = ctx.enter_context(tc.tile_pool(name="wts", bufs=1))
        opool = ctx.enter_context(tc.tile_pool(name="out", bufs=2))
        ppool = ctx.enter_context(tc.tile_pool(name="ps", bufs=8, space="PSUM"))

        CMID = IN_COLS // 2
        strips = []
        # Issue strip0 before the weights so the critical first window's
        # data transfer starts immediately; weights ride the idle sync queue.
        for wi, (in0, out0, kdim, m) in enumerate(WINDOWS):
            if wi == 0:
                st = spool.tile([128, IN_COLS], dtr, tag="strip")
                nc.gpsimd.dma_start(st[:kdim, :CMID], x[in0:in0 + kdim, :CMID])
                nc.scalar.dma_start(st[:kdim, CMID:], x[in0:in0 + kdim, CMID:])
                strips.append(st)
        wt = wpool.tile([128, 11 * M_OUT], dtr)
        nc.sync.dma_start(wt[:], wts[:])

        for wi, (in0, out0, kdim, m) in enumerate(WINDOWS):
            if wi == 0:
                st = strips[0]
            else:
                st = spool.tile([128, IN_COLS], dtr, tag="strip")
                nc.gpsimd.dma_start(st[:kdim, :CMID], x[in0:in0 + kdim, :CMID])
                nc.scalar.dma_start(st[:kdim, CMID:], x[in0:in0 + kdim, CMID:])
            ot = opool.tile([m, W], dtf, tag="out")
            for cc in range(NCHUNK):
                ps = ppool.tile([m, NCOL], dtf, tag="ps")
                for j, dj in enumerate(range(-MAXD, MAXD + 1)):
                    c0 = cc * NCOL + MAXD + dj
                    nc.tensor.matmul(
                        ps[:],
                        wt[:kdim, (dj + MAXD) * M_OUT:(dj + MAXD) * M_OUT + m],
                        st[:kdim, c0:c0 + NCOL],
                        start=(j == 0), stop=(j == 10),
                    )
                dst = ot[:, cc * NCOL:(cc + 1) * NCOL]
                if cc % 2 == 0:
                    nc.vector.tensor_copy(dst, ps[:])
                else:
                    nc.scalar.copy(dst, ps[:])
            # One fully-contiguous DRAM write per window (m full rows) so the
            # HW DGE fans it out across all 16 SDMA engines; keep stores off
            # the strip queues to avoid head-of-line blocking the prefetch.
            nc.sync.dma_start(y[out0:out0 + m, :], ot[:])
    nc.compile()
    return nc


def kernel(grid_spikes: np.ndarray, distance_weights: np.ndarray) -> np.ndarray:
    x = np.ascontiguousarray(grid_spikes, dtype=np.float32)
    assert x.shape == (H, W)
    if "nc" not in _CACHE:
        _CACHE["nc"] = _build()
    nc = _CACHE["nc"]

    w_flat = _band_weights(distance_weights)
    xpad = np.concatenate([x[:, -MAXD:], x, x[:, :MAXD]], axis=1)
    in_maps = []
    for c in range(N_CORES):
        rows = np.arange(c * ROWS_PER_CORE - MAXD,
                         c * ROWS_PER_CORE + ROWS_PER_CORE + MAXD) % H
        in_maps.append({"x": np.ascontiguousarray(xpad[rows]), "w": w_flat})

    res = run_bass_kernel_spmd(nc, in_maps, list(range(N_CORES)))
    out = np.concatenate([res.results[c]["y"] for c in range(N_CORES)], axis=0)
    return out.astype(np.float32)



# revision 5
# speedup vs baseline: 1.9421x; 1.9421x over previous
"""Trainium2 Bass kernel for nn_LocalConnectivity (diamond-ring circular stencil).

out[i,j] = sum_{d=1..5} w_d * sum_{|di|+|dj|=d} x[(i+di)%H, (j+dj)%W]

Strategy: 4x2 grid shard across 8 NeuronCores (1024x2048 block each + 5-wide
circular halo prepped on host, all IO in bf16).  Per core, 9 row-windows of
M<=118 output rows (K=M+10 input rows).  Per window the diamond stencil is
split by |column shift| j:
  - VectorE folds the symmetric column pairs sigma_j = x(c-j)+x(c+j) for
    j=1..4 (bf16 tensor_add at 2x mode; odd-offset operands are skewed by one
    column so every operand is 4B-aligned, the skew is absorbed by the matmul
    rhs offset).  Each sigma is built in left/right halves so the first two
    column chunks unblock early.
  - TensorE applies the vertical profiles as 7 banded bf16 matmuls per
    512-col chunk accumulating in one PSUM bank: the +-5 column shifts as two
    single-diagonal passes on x and V0 on x (11-tap band, center hole) go
    first (no sigma dependency), then V1..V4 on sigma_j (9/7/5/3-tap bands).
    bf16 streams ~1 cycle/col (vs ~2 for the old fp32r version).
  - ScalarE evicts PSUM->SBUF with the fp32->bf16 cast; outputs DMA out as
    bf16 and are cast/reassembled on host.
Engine separation: strip loads ride gpsimd (SWDGE), weights + stores ride
sync (HWDGE), ACT only evicts, DVE only folds.  A dozen dummy warm-up
matmuls on a memset scratch tile keep the PE HAM un-throttled through the
initial DMA wait.
"""
import numpy as np
import ml_dtypes
from contextlib import ExitStack

import concourse.bass as bass
import concourse.tile as tile
from concourse import bacc, mybir
from concourse.bass_utils import run_bass_kernel_spmd

N_CORES = 8
H = W = 4096
MAXD = 5
GRID_R, GRID_C = 4, 2                 # core grid
BR, BC = H // GRID_R, W // GRID_C     # 1024 x 2048 block per core
IN_ROWS = BR + 2 * MAXD               # 1034
IN_COLS = BC + 2 * MAXD               # 2058
NCOL = 512                            # matmul free dim (one PSUM bank, fp32)
NCHUNK = BC // NCOL                   # 4
M_OUT = 118                           # out rows per window (K=128-2*MAXD)
NPASS = 7                             # V5-, V0, V5+, V1..V4(sigma)
N_WARM = 14                           # HAM warm-up matmuls
# windows: (out_row_start, M, K)
WINDOWS = []
_o = 0
while _o < BR:
    m = min(M_OUT, BR - _o)
    WINDOWS.append((_o, m, m + 2 * MAXD))
    _o += m
# sigma skew: built start cols (5-j-s, 5+j-s) are even => 4B-aligned bf16
SKEW = {j: (MAXD - j) % 2 for j in range(1, MAXD)}
SIGW = BC + 2                         # sigma tile free width (even)
SHALF = SIGW // 2                     # left half [0,1026), right [1024,2050)

_CACHE = {}


def _band_weights(distance_weights: np.ndarray) -> np.ndarray:
    """[128, 7*118] bf16 stationary band matrices W_p[k, m] (d = k-m-5).

    p=0: single diagonal w[5] at d==0 (column shift -5)
    p=1: V0 band  w[|d|] for 1<=|d|<=5
    p=2: single diagonal w[5] at d==0 (column shift +5)
    p=3..6: V_j band (j=1..4)  w[j+|d|] for |d|<=5-j
    """
    wd = np.asarray(distance_weights, dtype=np.float32)
    out = np.zeros((NPASS, 128, M_OUT), dtype=np.float32)
    k = np.arange(128)[:, None]
    m = np.arange(M_OUT)[None, :]
    d = k - m - MAXD
    ad = np.abs(d)
    diag = np.where(d == 0, wd[MAXD - 1], 0.0)
    out[0] = diag
    out[1] = np.where((ad >= 1) & (ad <= MAXD), wd[np.minimum(ad, MAXD) - 1], 0.0)
    out[2] = diag
    for j in range(1, MAXD):
        out[2 + j] = np.where(ad <= MAXD - j,
                              wd[np.minimum(j + ad, MAXD) - 1], 0.0)
    flat = np.ascontiguousarray(out.transpose(1, 0, 2).reshape(128, NPASS * M_OUT))
    return flat.astype(ml_dtypes.bfloat16)


def _build():
    dtb = mybir.dt.bfloat16
    dtf = mybir.dt.float32
    nc = bacc.Bacc("TRN2", target_bir_lowering=False, debug=False,
                   num_devices=N_CORES)
    x = nc.dram_tensor("x", [IN_ROWS, IN_COLS], dtb, kind="ExternalInput").ap()
    wts = nc.dram_tensor("w", [128, NPASS * M_OUT], dtb,
                         kind="ExternalInput").ap()
    y = nc.dram_tensor("y", [BR, BC], dtb, kind="ExternalOutput").ap()

    with tile.TileContext(nc) as tc, ExitStack() as ctx:
        spool = ctx.enter_context(tc.tile_pool(name="strip", bufs=3))
        gpool = ctx.enter_context(tc.tile_pool(name="sig", bufs=3))
        wpool = ctx.enter_context(tc.tile_pool(name="wts", bufs=1))
        opool = ctx.enter_context(tc.tile_pool(name="out", bufs=3))
        ppool = ctx.enter_context(tc.tile_pool(name="ps", bufs=7, space="PSUM"))
        zpool = ctx.enter_context(tc.tile_pool(name="pw", bufs=1, space="PSUM"))

        # Weights first on the idle sync queue (they gate the first real MM),
        # then window 0's strip split across gpsimd + sync.
        wt = wpool.tile([128, NPASS * M_OUT], dtb)
        nc.sync.dma_start(wt[:], wts[:])
        CMID = IN_COLS // 2
        strips = {}
        o0, m0, k0 = WINDOWS[0]
        st = spool.tile([128, IN_COLS], dtb, tag="strip")
        nc.gpsimd.dma_start(st[:k0, :CMID], x[o0:o0 + k0, :CMID])
        nc.sync.dma_start(st[:k0, CMID:], x[o0:o0 + k0, CMID:])
        strips[0] = st

        # HAM warm-up: dummy matmuls on a zeroed scratch tile bridge the PE
        # from its preamble to the first data-gated matmul at full clock.
        zs = wpool.tile([128, M_OUT + NCOL], dtb, name="zs")
        nc.vector.memset(zs[:], 0)
        zp = zpool.tile([M_OUT, NCOL], dtf, tag="warm", name="zp")
        for _ in range(N_WARM):
            nc.tensor.matmul(zp[:], zs[:, :M_OUT], zs[:, M_OUT:],
                             start=True, stop=True)

        for wi, (out0, m, kdim) in enumerate(WINDOWS):
            if wi in strips:
                st = strips[wi]
            else:
                st = spool.tile([128, IN_COLS], dtb, tag="strip")
                nc.gpsimd.dma_start(st[:kdim, :], x[out0:out0 + kdim, :])
            # sigma_j (j=1..4) on VectorE in left/right halves
            sig = {}
            for j in range(1, MAXD):
                sig[j] = gpool.tile([128, SIGW], dtb, tag=f"sig{j}",
                                    name=f"sig{j}")
            for half in range(2):
                h0, h1 = (0, SHALF + 2) if half == 0 else (SHALF, SIGW)
                hw = h1 - h0
                for j in range(1, MAXD):
                    s = SKEW[j]
                    a0 = MAXD - j - s + h0
                    b0 = MAXD + j - s + h0
                    nc.vector.tensor_add(
                        sig[j][:kdim, h0:h1],
                        st[:kdim, a0:a0 + hw],
                        st[:kdim, b0:b0 + hw],
                    )
            ot = opool.tile([m, BC], dtb, tag="out")
            for cc in range(NCHUNK):
                c0 = cc * NCOL
                ps = ppool.tile([m, NCOL], dtf, tag="ps")
                # strip-only passes first, then the sigma passes
                rhs = [
                    st[:kdim, c0:c0 + NCOL],
                    st[:kdim, MAXD + c0:MAXD + c0 + NCOL],
                    st[:kdim, 2 * MAXD + c0:2 * MAXD + c0 + NCOL],
                ]
                for j in range(1, MAXD):
                    s = SKEW[j]
                    rhs.append(sig[j][:kdim, s + c0:s + c0 + NCOL])
                for p in range(NPASS):
                    nc.tensor.matmul(
                        ps[:],
                        wt[:kdim, p * M_OUT:p * M_OUT + m],
                        rhs[p],
                        start=(p == 0), stop=(p == NPASS - 1),
                    )
                nc.scalar.copy(ot[:, c0:c0 + NCOL], ps[:])
            nc.sync.dma_start(y[out0:out0 + m, :], ot[:])
    nc.compile()
    return nc


def _make_in_maps(grid_spikes: np.ndarray, distance_weights: np.ndarray):
    x = np.ascontiguousarray(grid_spikes, dtype=np.float32)
    assert x.shape == (H, W)
    w_flat = _band_weights(distance_weights)
    xpad = np.pad(x, MAXD, mode="wrap").astype(ml_dtypes.bfloat16)
    in_maps = []
    for c in range(N_CORES):
        rb, cb = divmod(c, GRID_C)
        strip = xpad[rb * BR:rb * BR + IN_ROWS, cb * BC:cb * BC + IN_COLS]
        in_maps.append({"x": np.ascontiguousarray(strip), "w": w_flat})
    return in_maps


def kernel(grid_spikes: np.ndarray, distance_weights: np.ndarray) -> np.ndarray:
    if "nc" not in _CACHE:
        _CACHE["nc"] = _build()
    nc = _CACHE["nc"]

    in_maps = _make_in_maps(grid_spikes, distance_weights)
    res = run_bass_kernel_spmd(nc, in_maps, list(range(N_CORES)))
    out = np.empty((H, W), dtype=np.float32)
    for c in range(N_CORES):
        rb, cb = divmod(c, GRID_C)
        out[rb * BR:(rb + 1) * BR, cb * BC:(cb + 1) * BC] = \
            res.results[c]["y"].astype(np.float32)
    return out


# revision 8
# speedup vs baseline: 2.6455x; 1.3622x over previous
"""Trainium2 Bass kernel for nn_LocalConnectivity (diamond-ring circular stencil).

out[i,j] = sum_{d=1..5} w_d * sum_{|di|+|dj|=d} x[(i+di)%H, (j+dj)%W]

Strategy: 4x2 grid shard across 8 NeuronCores (1024x2048 block each + 5-wide
circular halo prepped on host, all IO in bf16).  Per core, 9 row-windows of
M<=118 output rows (K=M+10 input rows).  Per window the diamond stencil is
split by |column shift| j:
  - VectorE folds the symmetric column pairs sigma_j = x(c-j)+x(c+j) for
    j=1..4 (bf16 tensor_add at 2x mode; odd-offset operands are skewed by one
    column so every operand is 4B-aligned, the skew is absorbed by the matmul
    rhs offset).  Each sigma is built in left/right halves so the first two
    column chunks unblock early.
  - TensorE applies the vertical profiles as 7 banded bf16 matmuls per
    512-col chunk accumulating in one PSUM bank: the +-5 column shifts as two
    single-diagonal passes on x and V0 on x (11-tap band, center hole) go
    first (no sigma dependency), then V1..V4 on sigma_j (9/7/5/3-tap bands).
    bf16 streams ~1 cycle/col (vs ~2 for the old fp32r version).
  - ScalarE evicts PSUM->SBUF with the fp32->bf16 cast; outputs DMA out as
    bf16 and are cast/reassembled on host.
Engine separation: strip loads + weights ride sync (HWDGE, which fans
HBM->SBUF loads across all 16 SDMA engines), stores ride gpsimd (SWDGE --
HWDGE stores collapse onto 2 SDMA engines), ACT only evicts, DVE only
folds.  A dozen dummy warm-up
matmuls on a memset scratch tile keep the PE HAM un-throttled through the
initial DMA wait.
"""
import numpy as np
import ml_dtypes
from contextlib import ExitStack

import concourse.bass as bass
import concourse.tile as tile
from concourse import bacc, mybir
from concourse.bass_utils import run_bass_kernel_spmd

N_CORES = 8
H = W = 4096
MAXD = 5
GRID_R, GRID_C = 4, 2                 # core grid
BR, BC = H // GRID_R, W // GRID_C     # 1024 x 2048 block per core
IN_ROWS = BR + 2 * MAXD               # 1034
IN_COLS = BC + 2 * MAXD               # 2058
NCOL = 512                            # matmul free dim (one PSUM bank, fp32)
NCHUNK = BC // NCOL                   # 4
M_OUT = 118                           # out rows per window (K=128-2*MAXD)
NPASS = 7                             # V5-, V0, V5+, V1..V4(sigma)
N_WARM = 14                           # HAM warm-up matmuls
# windows: (out_row_start, M, K)
WINDOWS = []
_o = 0
while _o < BR:
    m = min(M_OUT, BR - _o)
    WINDOWS.append((_o, m, m + 2 * MAXD))
    _o += m
# sigma skew: built start cols (5-j-s, 5+j-s) are even => 4B-aligned bf16
SKEW = {j: (MAXD - j) % 2 for j in range(1, MAXD)}
SIGW = BC + 2                         # sigma tile free width (even)
SHALF = SIGW // 2                     # left half [0,1026), right [1024,2050)

_CACHE = {}


def _band_weights(distance_weights: np.ndarray) -> np.ndarray:
    """[128, 7*118] bf16 stationary band matrices W_p[k, m] (d = k-m-5).

    p=0: single diagonal w[5] at d==0 (column shift -5)
    p=1: V0 band  w[|d|] for 1<=|d|<=5
    p=2: single diagonal w[5] at d==0 (column shift +5)
    p=3..6: V_j band (j=1..4)  w[j+|d|] for |d|<=5-j
    """
    wd = np.asarray(distance_weights, dtype=np.float32)
    out = np.zeros((NPASS, 128, M_OUT), dtype=np.float32)
    k = np.arange(128)[:, None]
    m = np.arange(M_OUT)[None, :]
    d = k - m - MAXD
    ad = np.abs(d)
    diag = np.where(d == 0, wd[MAXD - 1], 0.0)
    out[0] = diag
    out[1] = np.where((ad >= 1) & (ad <= MAXD), wd[np.minimum(ad, MAXD) - 1], 0.0)
    out[2] = diag
    for j in range(1, MAXD):
        out[2 + j] = np.where(ad <= MAXD - j,
                              wd[np.minimum(j + ad, MAXD) - 1], 0.0)
    flat = np.ascontiguousarray(out.transpose(1, 0, 2).reshape(128, NPASS * M_OUT))
    return flat.astype(ml_dtypes.bfloat16)


def _build():
    dtb = mybir.dt.bfloat16
    dtf = mybir.dt.float32
    nc = bacc.Bacc("TRN2", target_bir_lowering=False, debug=False,
                   num_devices=N_CORES)
    x = nc.dram_tensor("x", [IN_ROWS, IN_COLS], dtb, kind="ExternalInput").ap()
    wts = nc.dram_tensor("w", [128, NPASS * M_OUT], dtb,
                         kind="ExternalInput").ap()
    y = nc.dram_tensor("y", [BR, BC], dtb, kind="ExternalOutput").ap()

    with tile.TileContext(nc) as tc, ExitStack() as ctx:
        spool = ctx.enter_context(tc.tile_pool(name="strip", bufs=3))
        gpool = ctx.enter_context(tc.tile_pool(name="sig", bufs=3))
        wpool = ctx.enter_context(tc.tile_pool(name="wts", bufs=1))
        opool = ctx.enter_context(tc.tile_pool(name="out", bufs=3))
        ppool = ctx.enter_context(tc.tile_pool(name="ps", bufs=7, space="PSUM"))
        zpool = ctx.enter_context(tc.tile_pool(name="pw", bufs=1, space="PSUM"))

        # Weights first on the idle sync queue (they gate the first real MM),
        # then window 0's strip split across gpsimd + sync.
        wt = wpool.tile([128, NPASS * M_OUT], dtb)
        nc.sync.dma_start(wt[:], wts[:])
        CMID = IN_COLS // 2
        strips = {}
        o0, m0, k0 = WINDOWS[0]
        st = spool.tile([128, IN_COLS], dtb, tag="strip")
        nc.gpsimd.dma_start(st[:k0, :CMID], x[o0:o0 + k0, :CMID])
        nc.sync.dma_start(st[:k0, CMID:], x[o0:o0 + k0, CMID:])
        strips[0] = st
        # SBUF->HBM stores must ride SWDGE (gpsimd): HWDGE emits the
        # per-partition store descriptors onto only 2 of the 16 SDMA
        # engines (~50 GB/s); SWDGE's CounterMachine spreads all 16.

        # HAM warm-up: dummy matmuls on a zeroed scratch tile bridge the PE
        # from its preamble to the first data-gated matmul at full clock.
        zs = wpool.tile([128, M_OUT + NCOL], dtb, name="zs")
        nc.vector.memset(zs[:], 0)
        zp = zpool.tile([M_OUT, NCOL], dtf, tag="warm", name="zp")
        for _ in range(N_WARM):
            nc.tensor.matmul(zp[:], zs[:, :M_OUT], zs[:, M_OUT:],
                             start=True, stop=True)

        for wi, (out0, m, kdim) in enumerate(WINDOWS):
            if wi in strips:
                st = strips[wi]
            else:
                st = spool.tile([128, IN_COLS], dtb, tag="strip")
                nc.sync.dma_start(st[:kdim, :], x[out0:out0 + kdim, :])
            # sigma_j (j=1..4) on VectorE in left/right halves
            sig = {}
            for j in range(1, MAXD):
                sig[j] = gpool.tile([128, SIGW], dtb, tag=f"sig{j}",
                                    name=f"sig{j}")
            for half in range(2):
                h0, h1 = (0, SHALF + 2) if half == 0 else (SHALF, SIGW)
                hw = h1 - h0
                for j in range(1, MAXD):
                    s = SKEW[j]
                    a0 = MAXD - j - s + h0
                    b0 = MAXD + j - s + h0
                    nc.vector.tensor_add(
                        sig[j][:kdim, h0:h1],
                        st[:kdim, a0:a0 + hw],
                        st[:kdim, b0:b0 + hw],
                    )
            ot = opool.tile([m, BC], dtb, tag="out")
            for cc in range(NCHUNK):
                c0 = cc * NCOL
                ps = ppool.tile([m, NCOL], dtf, tag="ps")
                # strip-only passes first, then the sigma passes
                rhs = [
                    st[:kdim, c0:c0 + NCOL],
                    st[:kdim, MAXD + c0:MAXD + c0 + NCOL],
                    st[:kdim, 2 * MAXD + c0:2 * MAXD + c0 + NCOL],
                ]
                for j in range(1, MAXD):
                    s = SKEW[j]
                    rhs.append(sig[j][:kdim, s + c0:s + c0 + NCOL])
                for p in range(NPASS):
                    nc.tensor.matmul(
                        ps[:],
                        wt[:kdim, p * M_OUT:p * M_OUT + m],
                        rhs[p],
                        start=(p == 0), stop=(p == NPASS - 1),
                    )
                nc.scalar.copy(ot[:, c0:c0 + NCOL], ps[:])
            nc.gpsimd.dma_start(y[out0:out0 + m, :], ot[:])
    nc.compile()
    return nc


def _make_in_maps(grid_spikes: np.ndarray, distance_weights: np.ndarray):
    x = np.ascontiguousarray(grid_spikes, dtype=np.float32)
    assert x.shape == (H, W)
    w_flat = _band_weights(distance_weights)
    xpad = np.pad(x, MAXD, mode="wrap").astype(ml_dtypes.bfloat16)
    in_maps = []
    for c in range(N_CORES):
        rb, cb = divmod(c, GRID_C)
        strip = xpad[rb * BR:rb * BR + IN_ROWS, cb * BC:cb * BC + IN_COLS]
        in_maps.append({"x": np.ascontiguousarray(strip), "w": w_flat})
    return in_maps


def kernel(grid_spikes: np.ndarray, distance_weights: np.ndarray) -> np.ndarray:
    if "nc" not in _CACHE:
        _CACHE["nc"] = _build()
    nc = _CACHE["nc"]

    in_maps = _make_in_maps(grid_spikes, distance_weights)
    res = run_bass_kernel_spmd(nc, in_maps, list(range(N_CORES)))
    out = np.empty((H, W), dtype=np.float32)
    for c in range(N_CORES):
        rb, cb = divmod(c, GRID_C)
        out[rb * BR:(rb + 1) * BR, cb * BC:(cb + 1) * BC] = \
            res.results[c]["y"].astype(np.float32)
    return out


# revision 11
# speedup vs baseline: 2.6621x; 1.0063x over previous
"""Trainium2 Bass kernel for nn_LocalConnectivity (diamond-ring circular stencil).

out[i,j] = sum_{d=1..5} w_d * sum_{|di|+|dj|=d} x[(i+di)%H, (j+dj)%W]

Strategy: 4x2 grid shard across 8 NeuronCores (1024x2048 block each + 5-wide
circular halo prepped on host, all IO in bf16).  Per core, 9 row-windows of
M<=118 output rows (K=M+10 input rows).  Per window the diamond stencil is
split by |column shift| j:
  - VectorE folds the symmetric column pairs sigma_j = x(c-j)+x(c+j) for
    j=1..4 (bf16 tensor_add at 2x mode; odd-offset operands are skewed by one
    column so every operand is 4B-aligned, the skew is absorbed by the matmul
    rhs offset).  Each sigma is built in left/right halves so the first two
    column chunks unblock early.
  - TensorE applies the vertical profiles as 7 banded bf16 matmuls per
    512-col chunk accumulating in one PSUM bank: the +-5 column shifts as two
    single-diagonal passes on x and V0 on x (11-tap band, center hole) go
    first (no sigma dependency), then V1..V4 on sigma_j (9/7/5/3-tap bands).
    bf16 streams ~1 cycle/col (vs ~2 for the old fp32r version).
  - ScalarE evicts PSUM->SBUF with the fp32->bf16 cast; outputs DMA out as
    bf16 and are cast/reassembled on host.
Engine separation: strip loads + weights ride sync (HWDGE, which fans
HBM->SBUF loads across all 16 SDMA engines), stores ride gpsimd (SWDGE --
HWDGE stores collapse onto 2 SDMA engines), ACT only evicts, DVE only
folds.  A dozen dummy warm-up
matmuls on a memset scratch tile keep the PE HAM un-throttled through the
initial DMA wait.
"""
import numpy as np
import ml_dtypes
from contextlib import ExitStack

import concourse.bass as bass
import concourse.tile as tile
from concourse import bacc, mybir
from concourse.bass_utils import run_bass_kernel_spmd

N_CORES = 8
H = W = 4096
MAXD = 5
GRID_R, GRID_C = 4, 2                 # core grid
BR, BC = H // GRID_R, W // GRID_C     # 1024 x 2048 block per core
IN_ROWS = BR + 2 * MAXD               # 1034
IN_COLS = BC + 2 * MAXD               # 2058
NCOL = 512                            # matmul free dim (one PSUM bank, fp32)
NCHUNK = BC // NCOL                   # 4
M_OUT = 118                           # out rows per window (K=128-2*MAXD)
NPASS = 7                             # V5-, V0, V5+, V1..V4(sigma)
N_WARM = 9                            # HAM warm-up matmuls
# windows: (out_row_start, M, K)
WINDOWS = []
_o = 0
while _o < BR:
    m = min(M_OUT, BR - _o)
    WINDOWS.append((_o, m, m + 2 * MAXD))
    _o += m
# sigma skew: built start cols (5-j-s, 5+j-s) are even => 4B-aligned bf16
SKEW = {j: (MAXD - j) % 2 for j in range(1, MAXD)}
SIGW = BC + 2                         # sigma tile free width (even)
SHALF = SIGW // 2                     # left half [0,1026), right [1024,2050)

_CACHE = {}


def _band_weights(distance_weights: np.ndarray) -> np.ndarray:
    """[128, 7*118] bf16 stationary band matrices W_p[k, m] (d = k-m-5).

    p=0: single diagonal w[5] at d==0 (column shift -5)
    p=1: V0 band  w[|d|] for 1<=|d|<=5
    p=2: single diagonal w[5] at d==0 (column shift +5)
    p=3..6: V_j band (j=1..4)  w[j+|d|] for |d|<=5-j
    """
    wd = np.asarray(distance_weights, dtype=np.float32)
    out = np.zeros((NPASS, 128, M_OUT), dtype=np.float32)
    k = np.arange(128)[:, None]
    m = np.arange(M_OUT)[None, :]
    d = k - m - MAXD
    ad = np.abs(d)
    diag = np.where(d == 0, wd[MAXD - 1], 0.0)
    out[0] = diag
    out[1] = np.where((ad >= 1) & (ad <= MAXD), wd[np.minimum(ad, MAXD) - 1], 0.0)
    out[2] = diag
    for j in range(1, MAXD):
        out[2 + j] = np.where(ad <= MAXD - j,
                              wd[np.minimum(j + ad, MAXD) - 1], 0.0)
    flat = np.ascontiguousarray(out.transpose(1, 0, 2).reshape(128, NPASS * M_OUT))
    return flat.astype(ml_dtypes.bfloat16)


def _build():
    dtb = mybir.dt.bfloat16
    dtf = mybir.dt.float32
    nc = bacc.Bacc("TRN2", target_bir_lowering=False, debug=False,
                   num_devices=N_CORES)
    x = nc.dram_tensor("x", [IN_ROWS, IN_COLS], dtb, kind="ExternalInput").ap()
    wts = nc.dram_tensor("w", [128, NPASS * M_OUT], dtb,
                         kind="ExternalInput").ap()
    y = nc.dram_tensor("y", [BR, BC], dtb, kind="ExternalOutput").ap()

    with tile.TileContext(nc) as tc, ExitStack() as ctx:
        spool = ctx.enter_context(tc.tile_pool(name="strip", bufs=3))
        gpool = ctx.enter_context(tc.tile_pool(name="sig", bufs=3))
        wpool = ctx.enter_context(tc.tile_pool(name="wts", bufs=1))
        opool = ctx.enter_context(tc.tile_pool(name="out", bufs=3))
        ppool = ctx.enter_context(tc.tile_pool(name="ps", bufs=7, space="PSUM"))
        zpool = ctx.enter_context(tc.tile_pool(name="pw", bufs=1, space="PSUM"))

        # Weights first on the idle sync queue (they gate the first real MM),
        # then window 0's strip split across gpsimd + sync.
        wt = wpool.tile([128, NPASS * M_OUT], dtb)
        nc.sync.dma_start(wt[:], wts[:])
        CMID = IN_COLS // 2
        strips = {}
        o0, m0, k0 = WINDOWS[0]
        st = spool.tile([128, IN_COLS], dtb, tag="strip")
        nc.gpsimd.dma_start(st[:k0, :CMID], x[o0:o0 + k0, :CMID])
        nc.sync.dma_start(st[:k0, CMID:], x[o0:o0 + k0, CMID:])
        strips[0] = st
        # SBUF->HBM stores must ride SWDGE (gpsimd): HWDGE emits the
        # per-partition store descriptors onto only 2 of the 16 SDMA
        # engines (~50 GB/s); SWDGE's CounterMachine spreads all 16.

        # HAM warm-up: dummy matmuls on a zeroed scratch tile bridge the PE
        # from its preamble to the first data-gated matmul at full clock.
        zs = wpool.tile([128, M_OUT + NCOL], dtb, name="zs")
        nc.gpsimd.memset(zs[:], 0)
        zp = zpool.tile([M_OUT, NCOL], dtf, tag="warm", name="zp")
        for _ in range(N_WARM):
            nc.tensor.matmul(zp[:], zs[:, :M_OUT], zs[:, M_OUT:],
                             start=True, stop=True)

        for wi, (out0, m, kdim) in enumerate(WINDOWS):
            if wi in strips:
                st = strips[wi]
            else:
                st = spool.tile([128, IN_COLS], dtb, tag="strip")
                nc.sync.dma_start(st[:kdim, :], x[out0:out0 + kdim, :])
            # sigma_j (j=1..4) on VectorE in left/right halves
            sig = {}
            for j in range(1, MAXD):
                sig[j] = gpool.tile([128, SIGW], dtb, tag=f"sig{j}",
                                    name=f"sig{j}")
            for half in range(2):
                h0, h1 = (0, SHALF + 2) if half == 0 else (SHALF, SIGW)
                hw = h1 - h0
                for j in range(1, MAXD):
                    s = SKEW[j]
                    a0 = MAXD - j - s + h0
                    b0 = MAXD + j - s + h0
                    nc.vector.tensor_add(
                        sig[j][:kdim, h0:h1],
                        st[:kdim, a0:a0 + hw],
                        st[:kdim, b0:b0 + hw],
                    )
            ot = opool.tile([m, BC], dtb, tag="out")
            for cc in range(NCHUNK):
                c0 = cc * NCOL
                ps = ppool.tile([m, NCOL], dtf, tag="ps")
                # strip-only passes first, then the sigma passes
                rhs = [
                    st[:kdim, c0:c0 + NCOL],
                    st[:kdim, MAXD + c0:MAXD + c0 + NCOL],
                    st[:kdim, 2 * MAXD + c0:2 * MAXD + c0 + NCOL],
                ]
                for j in range(1, MAXD):
                    s = SKEW[j]
                    rhs.append(sig[j][:kdim, s + c0:s + c0 + NCOL])
                for p in range(NPASS):
                    nc.tensor.matmul(
                        ps[:],
                        wt[:kdim, p * M_OUT:p * M_OUT + m],
                        rhs[p],
                        start=(p == 0), stop=(p == NPASS - 1),
                    )
                last = wi == len(WINDOWS) - 1
                # Last window: alternate evict engines and store per chunk so
                # the drain of the final stores overlaps the final matmuls.
                if last and cc % 2 == 1:
                    nc.vector.tensor_copy(ot[:, c0:c0 + NCOL], ps[:])
                else:
                    nc.scalar.copy(ot[:, c0:c0 + NCOL], ps[:])
                if last:
                    nc.gpsimd.dma_start(y[out0:out0 + m, c0:c0 + NCOL],
                                        ot[:, c0:c0 + NCOL])
            if not last:
                nc.gpsimd.dma_start(y[out0:out0 + m, :], ot[:])
    nc.compile()
    return nc


def _make_in_maps(grid_spikes: np.ndarray, distance_weights: np.ndarray):
    x = np.ascontiguousarray(grid_spikes, dtype=np.float32)
    assert x.shape == (H, W)
    w_flat = _band_weights(distance_weights)
    xpad = np.pad(x, MAXD, mode="wrap").astype(ml_dtypes.bfloat16)
    in_maps = []
    for c in range(N_CORES):
        rb, cb = divmod(c, GRID_C)
        strip = xpad[rb * BR:rb * BR + IN_ROWS, cb * BC:cb * BC + IN_COLS]
        in_maps.append({"x": np.ascontiguousarray(strip), "w": w_flat})
    return in_maps


def kernel(grid_spikes: np.ndarray, distance_weights: np.ndarray) -> np.ndarray:
    if "nc" not in _CACHE:
        _CACHE["nc"] = _build()
    nc = _CACHE["nc"]

    in_maps = _make_in_maps(grid_spikes, distance_weights)
    res = run_bass_kernel_spmd(nc, in_maps, list(range(N_CORES)))
    out = np.empty((H, W), dtype=np.float32)
    for c in range(N_CORES):
        rb, cb = divmod(c, GRID_C)
        out[rb * BR:(rb + 1) * BR, cb * BC:(cb + 1) * BC] = \
            res.results[c]["y"].astype(np.float32)
    return out
